# revision 22
# baseline (speedup 1.0000x reference)
"""Trainium2 Bass kernel for nn_ContrastiveLoss (N=8192, D=256), 8 NeuronCores.

Strategy (data-parallel over query rows, no collectives):
  - Each core receives the FULL x, y [8192, 256] fp32 plus its own 1024-row
    query slices qx, qy.  Host sums the 8 partial scalars.
  - On-core: row norms via bn_stats/bn_aggr (m = mean(x^2); ss = D*m);
    inv = m**-0.5 = exp(-0.5*ln(m)) -- Ln and Exp live in one ACT table set
    (natural_log_exp_and_others; forced via the act-table map so the set is
    loaded exactly once).  The 1e-8 eps of the reference shifts results by
    ~6e-10 relative - far below fp32 noise.
  - Rows normalized in natural layout with one fused tensor_scalar
    (x * inv_row * (1/sqrt(D))) -> bf16, then transposed 128x128-wise on the
    PE (transpose mode, bf16 in/out) into keysT [256 x 8192] (d on
    partitions), copied PSUM->SBUF by the DVE.
  - Stage B: sim row-stripes [128q, 2048keys] accumulated in PSUM fp32 from
    bf16 matmuls (contraction d=256 split in 2 psum-accumulated chunks);
    fused exp+row-sum on the Scalar engine (activation Exp with accum_out),
    exp output discarded in-place in PSUM.
  - logsumexp = Ln(sum of stripe row sums); pos terms: pos_xx = pos_yy = 1
    exactly (up to eps), pos_xy from fp32 dot products in natural layout.

Self-contained: only needs numpy + the /opt/trn_rl_repo concourse stack.
"""

import sys

for _p in ("/opt/trn_rl_repo", "/root/.axon_site/_ro/trn_rl_repo"):
    if _p not in sys.path:
        sys.path.insert(0, _p)

import numpy as np

import concourse.bass as bass
import concourse.mybir as mybir
import concourse.tile as tile
from concourse import bacc

FP32 = mybir.dt.float32
BF16 = mybir.dt.bfloat16
AX = mybir.AxisListType
AOP = mybir.AluOpType
AF = mybir.ActivationFunctionType

N, D = 8192, 256
NCORES = 8
P = 128
QR = N // NCORES          # 1024 query rows per core
QTILES = QR // P          # 8 query tiles
NST = 4                   # key chunks (2048 rows each)
GPC = (N // NST) // P     # 16 row-tiles per chunk
DC = D // P               # 2 contraction chunks of 128
NSIM = 3                  # xx, xy, yy
STRIPE = 2048             # stage-B free width (4 PSUM banks)
RS_COLS = NSIM * QTILES * (N // STRIPE)   # 24 * 4 = 96


def _force_single_act_table():
    """Make bacc's act-table fixpoint choose natural_log_exp_and_others for
    Exp/Ln/Copy so the kernel does exactly one ACT_TABLE_LOAD."""
    if getattr(bacc, "_contrastive_tables_patched", False):
        return
    orig = bacc.get_activation_tables
    keep = "natural_log_exp_and_others"
    ours = {AF.Exp, AF.Ln, AF.Copy, AF.Identity}

    def patched(arch):
        tabs = orig(arch)
        if keep not in tabs:
            return tabs
        return {
            name: (funcs if name == keep else set(funcs) - ours)
            for name, funcs in tabs.items()
        }

    patched.__wrapped__ = orig
    bacc.get_activation_tables = patched
    bacc._contrastive_tables_patched = True


def _build_program():
    _force_single_act_table()
    nc = bacc.Bacc("TRN2", target_bir_lowering=False, debug=False)
    x_d = nc.dram_tensor("x", [N, D], FP32, kind="ExternalInput").ap()
    y_d = nc.dram_tensor("y", [N, D], FP32, kind="ExternalInput").ap()
    qx_d = nc.dram_tensor("qx", [QR, D], FP32, kind="ExternalInput").ap()
    qy_d = nc.dram_tensor("qy", [QR, D], FP32, kind="ExternalInput").ap()
    out_d = nc.dram_tensor("out", [P, 32], FP32, kind="ExternalOutput").ap()

    with tile.TileContext(nc) as tc:
        _emit(nc, tc, x_d, y_d, qx_d, qy_d, out_d)
    nc.compile()
    return nc


def _emit(nc, tc, x_d, y_d, qx_d, qy_d, out_d):
    from contextlib import ExitStack

    ctx = ExitStack()
    with ctx:
        singles = ctx.enter_context(tc.tile_pool(name="singles", bufs=1))
        natp = ctx.enter_context(tc.tile_pool(name="natp", bufs=2))
        convp = ctx.enter_context(tc.tile_pool(name="convp", bufs=2))
        smallp = ctx.enter_context(tc.tile_pool(name="smallp", bufs=3))
        psX = ctx.enter_context(tc.tile_pool(name="psX", bufs=2, space="PSUM"))

        # bf16 identity for PE transpose-mode
        eyeb = singles.tile([P, P], BF16, tag="eyeb")
        nc.gpsimd.memset(eyeb, 0.0)
        nc.gpsimd.affine_select(
            out=eyeb, in_=eyeb, compare_op=AOP.not_equal, fill=1.0,
            base=0, pattern=[[-1, P]], channel_multiplier=1)

        # persistent transposed-normalized key/query tiles (bf16)
        xnT = [[singles.tile([P, 2048], BF16, tag=f"xnT{c}_{st}",
                             name=f"xnT{c}_{st}")
                for st in range(NST)] for c in range(DC)]
        ynT = [[singles.tile([P, 2048], BF16, tag=f"ynT{c}_{st}",
                             name=f"ynT{c}_{st}")
                for st in range(NST)] for c in range(DC)]
        qxT = [singles.tile([P, QR], BF16, tag=f"qxT{c}", name=f"qxT{c}")
               for c in range(DC)]
        qyT = [singles.tile([P, QR], BF16, tag=f"qyT{c}", name=f"qyT{c}")
               for c in range(DC)]
        rs = singles.tile([P, RS_COLS], FP32, tag="rs")

        def load_nat(dram, r0, ntiles, tag, bufs=2):
            t = natp.tile([P, ntiles, D], FP32, tag=tag, name=tag, bufs=bufs)
            src = dram[r0:r0 + ntiles * P, :].rearrange("(g p) d -> p g d", p=P)
            nc.sync.dma_start(out=t, in_=src)
            return t

        def row_inv_act(nat, ntiles, tag, pool=None):
            """inv = 1/||row|| via ACT Square+accum (for the prologue where
            ACT is otherwise idle); returns TRUE inverse norms [P, ntiles]."""
            ss = smallp.tile([P, ntiles], FP32, tag=tag + "_ss",
                             name=tag + "_ss")
            for g in range(ntiles):
                sq = smallp.tile([P, D], FP32, tag="sq", name="sq", bufs=2)
                nc.scalar.activation(sq, nat[:, g, :], AF.Square,
                                     accum_out=ss[:, g:g + 1])
            lnm = smallp.tile([P, ntiles], FP32, tag=tag + "_ln",
                              name=tag + "_ln")
            nc.scalar.activation(lnm, ss, AF.Ln)
            pool = pool or smallp
            inv = pool.tile([P, ntiles], FP32, tag=tag, name=tag)
            nc.scalar.activation(inv, lnm, AF.Exp, scale=-0.5)
            return inv, 1.0

        def row_inv_dve(nat, ntiles, tag, pool=None):
            """inv = mean(row^2)**-0.5 = sqrt(D)/||row||  -> [P, ntiles];
            scale 1/sqrt(D) folded into the normalize step."""
            mv = smallp.tile([P, ntiles, 2], FP32, tag=tag + "_mv",
                             name=tag + "_mv")
            for g in range(ntiles):
                stats = smallp.tile([P, 6], FP32, tag="stats", name="stats",
                                    bufs=4)
                nc.vector.bn_stats(out=stats, in_=nat[:, g, :])
                nc.vector.bn_aggr(out=mv[:, g, :], in_=stats)
            m = smallp.tile([P, ntiles], FP32, tag=tag + "_m", name=tag + "_m")
            nc.vector.tensor_mul(m, mv[:, :, 0], mv[:, :, 0])
            nc.vector.tensor_add(m, m, mv[:, :, 1])
            lnm = smallp.tile([P, ntiles], FP32, tag=tag + "_ln",
                              name=tag + "_ln")
            nc.scalar.activation(lnm, m, AF.Ln)
            pool = pool or smallp
            inv = pool.tile([P, ntiles], FP32, tag=tag, name=tag)
            nc.scalar.activation(inv, lnm, AF.Exp, scale=-0.5)
            return inv, 1.0 / 16.0

        def xform(nat, inv_s, ntiles, dstT, tag):
            """dstT[c][:, 0:ntiles*128] = (rows normalized nat)^T in bf16"""
            inv, s2 = inv_s
            cb = convp.tile([P, ntiles, D], BF16, tag=tag, name=tag)
            for g in range(ntiles):
                if s2 == 1.0:
                    nc.vector.tensor_scalar_mul(
                        cb[:, g, :], nat[:, g, :], inv[:, g:g + 1])
                else:
                    nc.vector.tensor_scalar(
                        out=cb[:, g, :], in0=nat[:, g, :],
                        scalar1=inv[:, g:g + 1], scalar2=s2,
                        op0=AOP.mult, op1=AOP.mult)
            pb = psX.tile([P, DC * ntiles * P], BF16, tag="big", name="pbA")
            for c in range(DC):
                for g in range(ntiles):
                    nc.tensor.matmul(
                        pb[:, c * ntiles * P + g * P:
                           c * ntiles * P + (g + 1) * P],
                        lhsT=cb[:, g, c * P:(c + 1) * P],
                        rhs=eyeb, is_transpose=True, start=True, stop=True)
            for c in range(DC):
                nc.vector.tensor_copy(
                    dstT[c][:, 0:ntiles * P],
                    pb[:, c * ntiles * P:(c + 1) * ntiles * P])

        # ---------------- prologue ----------------
        nats = {}

        def load_chunk(st):
            nats[("x", st)] = load_nat(x_d, st * 2048, GPC, "natx")
            nats[("y", st)] = load_nat(y_d, st * 2048, GPC, "naty")

        nats[("x", 0)] = load_nat(x_d, 0, GPC, "natx")
        qxn = load_nat(qx_d, 0, QTILES, "qxn", bufs=1)
        qyn = load_nat(qy_d, 0, QTILES, "qyn", bufs=1)
        nats[("y", 0)] = load_nat(y_d, 0, GPC, "naty")
        load_chunk(1)

        inv_x0 = row_inv_act(nats[("x", 0)], GPC, "inv_x0")
        inv_qx = row_inv_act(qxn, QTILES, "inv_qx", pool=singles)
        inv_qy = row_inv_act(qyn, QTILES, "inv_qy", pool=singles)
        xform(nats.pop(("x", 0)), inv_x0, GPC,
              [xnT[c][0] for c in range(DC)], "kcb")
        xform(qxn, inv_qx, QTILES, qxT, "qcb")
        inv_y0 = row_inv_act(nats[("y", 0)], GPC, "inv_y0")
        xform(qyn, inv_qy, QTILES, qyT, "qcb")
        xform(nats.pop(("y", 0)), inv_y0, GPC,
              [ynT[c][0] for c in range(DC)], "kcb")

        # ---------------- main loop ----------------
        def stripes(st, sims):
            for sim_i, qT, kT in sims:
                for qt in range(QTILES):
                    pb = psX.tile([P, STRIPE], FP32, tag="big", name="pbB")
                    for nb in range(STRIPE // 512):
                        for c in range(DC):
                            nc.tensor.matmul(
                                pb[:, nb * 512:(nb + 1) * 512],
                                lhsT=qT[c][:, qt * P:(qt + 1) * P],
                                rhs=kT[c][st][:, nb * 512:(nb + 1) * 512],
                                start=(c == 0), stop=(c == DC - 1))
                    col = (sim_i * QTILES + qt) * NST + st
                    nc.scalar.activation(
                        pb, pb, AF.Exp, accum_out=rs[:, col:col + 1])

        def stripes_one(st, sim_i, qT, kT, qt_range):
            for qt in qt_range:
                pb = psX.tile([P, STRIPE], FP32, tag="big", name="pbB")
                for nb in range(STRIPE // 512):
                    for c in range(DC):
                        nc.tensor.matmul(
                            pb[:, nb * 512:(nb + 1) * 512],
                            lhsT=qT[c][:, qt * P:(qt + 1) * P],
                            rhs=kT[c][st][:, nb * 512:(nb + 1) * 512],
                            start=(c == 0), stop=(c == DC - 1))
                col = (sim_i * QTILES + qt) * NST + st
                nc.scalar.activation(
                    pb, pb, AF.Exp, accum_out=rs[:, col:col + 1])

        for st in range(NST):
            # xx then xy: only xnT/ynT of this chunk + qxT needed
            stripes(st, [(0, qxT, xnT), (1, qxT, ynT)])
            if st + 1 < NST:
                # prep next chunk's norms while this chunk's yy runs
                invs = {}
                for tname in ("x", "y"):
                    invs[tname] = row_inv_dve(
                        nats[(tname, st + 1)], GPC, f"inv_{tname}")
            # yy stripes, with next chunk's transposes slotted in between so
            # the chunk boundary has no ACT bubble
            stripes_one(st, 2, qyT, ynT, range(0, 3))
            if st + 1 < NST:
                xform(nats.pop(("x", st + 1)), invs["x"], GPC,
                      [xnT[c][st + 1] for c in range(DC)], "kcb")
            stripes_one(st, 2, qyT, ynT, range(3, QTILES))
            if st + 1 < NST:
                xform(nats.pop(("y", st + 1)), invs["y"], GPC,
                      [ynT[c][st + 1] for c in range(DC)], "kcb")
                if st + 2 < NST:
                    load_chunk(st + 2)

        # pos2 = (qx . qy) * inv_qx * inv_qy   [P, QTILES] (true inverses)
        dotxy = singles.tile([P, QTILES], FP32, tag="dotxy")
        for g in range(QTILES):
            sq = smallp.tile([P, D], FP32, tag="sqd", name="sqd", bufs=2)
            nc.vector.tensor_mul(sq, qxn[:, g, :], qyn[:, g, :])
            nc.vector.reduce_sum(out=dotxy[:, g:g + 1], in_=sq, axis=AX.X)
        pos2 = singles.tile([P, QTILES], FP32, tag="pos2")
        nc.vector.tensor_mul(pos2, dotxy, inv_qx[0])
        nc.vector.tensor_mul(pos2, pos2, inv_qy[0])

        # ---------------- epilogue ----------------
        rsum = singles.tile([P, NSIM * QTILES], FP32, tag="rsum")
        nc.vector.reduce_sum(
            out=rsum, in_=rs.rearrange("p (a b) -> p a b", b=NST),
            axis=AX.X)
        lse = singles.tile([P, NSIM * QTILES], FP32, tag="lse")
        nc.scalar.activation(lse, rsum, AF.Ln)
        nc.sync.dma_start(out=out_d[:, 0:NSIM * QTILES], in_=lse)
        nc.sync.dma_start(out=out_d[:, 24:24 + QTILES], in_=pos2)


_STATE = {}


def _get_state():
    if "nc" not in _STATE:
        _STATE["nc"] = _build_program()
    return _STATE["nc"]


class _Exec:
    """Persistent jitted multi-core executor (mirrors the multi-core path of
    bass2jax.run_bass_via_pjrt, but compiled once and reused)."""

    def __init__(self, nc):
        import jax
        import numpy as _np
        from jax.sharding import Mesh, PartitionSpec
        from jax.experimental.shard_map import shard_map
        from concourse import bass2jax, mybir as _mybir
        bass2jax.install_neuronx_cc_hook()
        self.jax = jax
        partition_name = (nc.partition_id_tensor.name
                          if nc.partition_id_tensor else None)
        in_names, out_names, out_avals, zero_outs = [], [], [], []
        for alloc in nc.m.functions[0].allocations:
            if not isinstance(alloc, _mybir.MemoryLocationSet):
                continue
            name = alloc.memorylocations[0].name
            if alloc.kind == "ExternalInput":
                if name != partition_name:
                    in_names.append(name)
            elif alloc.kind == "ExternalOutput":
                shape = tuple(alloc.tensor_shape)
                dtype = _mybir.dt.np(alloc.dtype)
                out_names.append(name)
                out_avals.append(jax.core.ShapedArray(shape, dtype))
                zero_outs.append(_np.zeros(shape, dtype))
        self.in_names = list(in_names)
        self.out_names = out_names
        self.zero_outs = zero_outs
        n_params = len(in_names)
        n_outs = len(out_avals)
        all_in_names = in_names + out_names
        if partition_name is not None:
            all_in_names = all_in_names + [partition_name]

        def _body(*args):
            operands = list(args)
            if partition_name is not None:
                operands.append(bass2jax.partition_id_tensor())
            outs = bass2jax._bass_exec_p.bind(
                *operands,
                out_avals=tuple(out_avals),
                in_names=tuple(all_in_names),
                out_names=tuple(out_names),
                lowering_input_output_aliases=(),
                sim_require_finite=True,
                sim_require_nnan=True,
                nc=nc,
            )
            return tuple(outs)

        devices = jax.devices()[:NCORES]
        self.mesh = Mesh(_np.asarray(devices), ("core",))
        in_specs = (PartitionSpec("core"),) * (n_params + n_outs)
        out_specs = (PartitionSpec("core"),) * n_outs
        self.sharded = jax.jit(
            shard_map(_body, mesh=self.mesh, in_specs=in_specs,
                      out_specs=out_specs, check_rep=False),
            donate_argnums=tuple(range(n_params, n_params + n_outs)),
            keep_unused=True,
        )

    def concat_inputs(self, in_maps):
        import numpy as _np
        return [
            _np.concatenate([_np.asarray(m[name]) for m in in_maps], axis=0)
            for name in self.in_names
        ]

    def run_concat(self, concat_in):
        import numpy as _np
        zouts = [
            _np.concatenate([z] * NCORES, axis=0) for z in self.zero_outs
        ]
        outs = self.sharded(*concat_in, *zouts)
        return outs

    def split(self, outs):
        import numpy as _np
        res = []
        arrs = [_np.asarray(o) for o in outs]
        for c in range(NCORES):
            res.append({
                name: arrs[i][c * arrs[i].shape[0] // NCORES:
                              (c + 1) * arrs[i].shape[0] // NCORES]
                for i, name in enumerate(self.out_names)
            })
        return res

    def run(self, in_maps):
        return self.split(self.run_concat(self.concat_inputs(in_maps)))


def _get_exec():
    if "exec" not in _STATE:
        _STATE["exec"] = _Exec(_get_state())
    return _STATE["exec"]


class _Res:
    def __init__(self, results):
        self.results = results
        self.exec_time_ns = None


def _run_on_hw(in_maps, trace=False, **kw):
    if trace:
        from concourse import bass_utils
        nc = _get_state()
        return bass_utils.run_bass_kernel_spmd(
            nc, in_maps, core_ids=list(range(NCORES)), trace=True, **kw)
    return _Res(_get_exec().run(in_maps))


def _make_in_maps(x, y):
    x = np.ascontiguousarray(x, dtype=np.float32)
    y = np.ascontiguousarray(y, dtype=np.float32)
    in_maps = []
    for c in range(NCORES):
        in_maps.append({
            "x": x, "y": y,
            "qx": np.ascontiguousarray(x[c * QR:(c + 1) * QR]),
            "qy": np.ascontiguousarray(y[c * QR:(c + 1) * QR]),
        })
    return in_maps


def _finish(outs):
    """outs: list of per-core {'out': [128, 32]} -> scalar loss"""
    total = 0.0
    for o in outs:
        arr = np.asarray(o["out"], dtype=np.float64)
        lse = arr[:, 0:NSIM * QTILES]
        pos2 = arr[:, 24:24 + QTILES]
        total += lse.sum() - pos2.sum() - 2.0 * QR
    return np.float32(total)


def kernel(x: np.ndarray, y: np.ndarray) -> np.ndarray:
    res = _run_on_hw(_make_in_maps(x, y), trace=False)
    return np.asarray(_finish(res.results), dtype=np.float32)
